# revision 18
# baseline (speedup 1.0000x reference)
"""Trainium2 Bass kernel for nn_AttentionBlock (smooth-softmax attention).

  out = smoothsoftmax((x@Wq+bq) @ (y@Wk+bk)^T) @ (y@Wv+bv)
  smoothsoftmax(M) = (0.1*relu(M) + softmax(M)) / rowsum(...)

Strategy (per core, x row-sharded across 8 cores):
  - Everything kept transposed on-chip: M^T chunks [128 keys, R rows] so both
    the softmax denominator and the output contraction over the 4096 key dim
    become PE matmul accumulations (no large transposes, no cross-partition
    reductions).
  - exp without max-subtraction (|M| <= ~20 so fp32 exp is safe).
  - out_i = (0.1*P1_i + P2_i/S_i) / (1 + 0.1*R_i) with
      P1 = relu(M) @ YV, P2 = e^M @ YV, S = rowsum(e^M), R = rowsum(relu(M));
    S and R come free as a ones-column appended to YV.
"""

import numpy as np
from contextlib import ExitStack

import concourse.bass as bass
import concourse.mybir as mybir
import concourse.tile as tile
from concourse.masks import make_identity

# ----------------------------------------------------------------------------
# Workaround for walrus "Too many sync wait commands" on the TileContext
# kernel-tail Drain: pre-issue the global-clock waits on the sync engine one
# per nop before the drain; the drain itself then needs no waits (SP executes
# in order).
from concourse.vector_clock import ScopedClock, VectorClock


def _drain_and_barrier_split(self, tick_clock, wait_clock):
    gc = tick_clock.global_clock
    n = len(gc)
    procs = [p for p in range(n) if gc[p] > 0]
    for p in procs:
        vec = [gc[q] if q == p else 0 for q in range(n)]
        nop = self.nc.sync.nop(nofuse=True, hint="drain_wait_split")
        wait_clock.add_sem_waits(nop.ins, ScopedClock({None: VectorClock(vec)}))
    self.nc.sync.drain()
    self.nc.all_engine_barrier()
    assert self.sems is not None
    popped = self.nc._tile_sem_poison_stack.pop()
    assert popped is self._sem_poison
    self.nc.clear_and_free_semaphores(list(self.sems.allocated().values()))
    self.nc.all_engine_barrier()


tile.TileContext._drain_and_barrier = _drain_and_barrier_split

from concourse import bass_utils as _bu

_orig_run_command = _bu.run_command


def _run_command_ldwopt(argv, **kwargs):
    argv = list(argv)
    return _orig_run_command(argv, **kwargs)


_bu.run_command = _run_command_ldwopt


def _split_multi_waits(nc, max_waits=1):
    """This walrus build rejects instructions carrying more than one sync
    wait.  Hoist extra waits onto single-wait NoOps on the same engine
    immediately before the instruction (engine streams execute in order,
    so semantics are identical)."""
    for f in nc.m.functions:
        for b in f.blocks:
            out = []
            changed = False
            for inst in b.instructions:
                si = inst.sync_info
                if si is not None and si.on_wait and len(si.on_wait) > max_waits:
                    waits = list(si.on_wait)
                    for w in waits[max_waits:]:
                        out.append(mybir.InstNoOp(
                            name=nc.get_next_instruction_name(),
                            engine=inst.engine,
                            bass_nofuse=True,
                            sync_info=mybir.SyncInfo(on_wait=[w], on_update=[]),
                        ))
                    si.on_wait = waits[:max_waits]
                    changed = True
                out.append(inst)
            if changed:
                b.instructions = out
# ----------------------------------------------------------------------------

F32 = mybir.dt.float32
F32R = mybir.dt.float32r
BF16 = mybir.dt.bfloat16

N_CORES = 8
N_FULL = 50000
S_IN = 256
NY = 4096
YDIM = 7
D = 64

ROWS_PER_CORE = (N_FULL + N_CORES - 1) // N_CORES  # 6250
PAD_ROWS = ((ROWS_PER_CORE + 127) // 128) * 128    # 6272

AF = mybir.ActivationFunctionType
ALU = mybir.AluOpType


def build_nc(pad_rows=PAD_ROWS, ny=NY, big_tile=512, split_waits=True):
    """Build the per-core Bass program. All 8 cores run the same program on
    different x shards (y and the projection weights are replicated)."""
    nc = bass.Bass(trn_type="TRN2")

    xs_h = nc.dram_tensor("xs", [pad_rows, S_IN], F32, kind="ExternalInput")
    y_h = nc.dram_tensor("y", [ny, YDIM], F32, kind="ExternalInput")
    wq_h = nc.dram_tensor("Wq", [S_IN, D], F32, kind="ExternalInput")
    bq_h = nc.dram_tensor("bq", [D], F32, kind="ExternalInput")
    wk_h = nc.dram_tensor("Wk", [YDIM, D], F32, kind="ExternalInput")
    bk_h = nc.dram_tensor("bk", [D], F32, kind="ExternalInput")
    wv_h = nc.dram_tensor("Wv", [YDIM, D], F32, kind="ExternalInput")
    bv_h = nc.dram_tensor("bv", [D], F32, kind="ExternalInput")
    out_h = nc.dram_tensor("out", [pad_rows, D], F32, kind="ExternalOutput")

    nchunks = ny // 128          # key chunks of 128
    assert ny % 512 == 0

    # row tiles: big_tile-row tiles then a 128-multiple remainder tile
    tiles = []
    r0 = 0
    while r0 + big_tile <= pad_rows:
        tiles.append((r0, big_tile))
        r0 += big_tile
    if r0 < pad_rows:
        assert (pad_rows - r0) % 128 == 0
        tiles.append((r0, pad_rows - r0))

    GROUP_W = 1024  # fp32 elems per partition in one m-psum slot (2 banks)

    with tile.TileContext(nc) as tc, ExitStack() as ctx:
        singles = ctx.enter_context(tc.tile_pool(name="singles", bufs=1))
        psum_m = ctx.enter_context(tc.tile_pool(name="psum_m", bufs=2, space="PSUM"))
        psum_acc = ctx.enter_context(tc.tile_pool(name="psum_acc", bufs=2, space="PSUM"))
        xin_pool = ctx.enter_context(tc.tile_pool(name="xin", bufs=2))
        xt_pool = ctx.enter_context(tc.tile_pool(name="xt", bufs=2))
        xq_pool = ctx.enter_context(tc.tile_pool(name="xq", bufs=2))
        er_pool = ctx.enter_context(tc.tile_pool(name="er", bufs=4))
        tail_pool = ctx.enter_context(tc.tile_pool(name="tail", bufs=2))

        # ------------------------------------------------------------------
        # Constants / precompute (once per core)
        # ------------------------------------------------------------------
        ident = singles.tile([128, 128], F32)
        make_identity(nc, ident)

        # y^T with a ones row appended: yTe [8, ny]
        ys = singles.tile([128, nchunks, YDIM], F32)
        nc.sync.dma_start(out=ys, in_=y_h[:, :].rearrange("(c p) d -> p c d", p=128))
        yTe = singles.tile([YDIM, ny], BF16)
        assert nchunks % 4 == 0
        for g in range(nchunks // 4):
            yt_ps = psum_m.tile([8, 512], F32, tag="m")
            for k in range(4):
                c = g * 4 + k
                nc.tensor.transpose(
                    yt_ps[0:YDIM, k * 128:(k + 1) * 128], ys[:, c, :], ident
                )
            nc.vector.tensor_copy(
                out=yTe[0:YDIM, g * 512:(g + 1) * 512], in_=yt_ps[0:YDIM, :]
            )

        # [Wk; bk] and [Wv; bv] stacked: [8, D]
        wkb_f = singles.tile([YDIM, D], F32)
        nc.sync.dma_start(out=wkb_f, in_=wk_h[:, :])
        wkb = singles.tile([YDIM, D], BF16)
        nc.vector.tensor_copy(out=wkb, in_=wkb_f)
        wvb_f = singles.tile([YDIM, D], F32)
        nc.sync.dma_start(out=wvb_f, in_=wv_h[:, :])
        wvb = singles.tile([YDIM, D], BF16)
        nc.vector.tensor_copy(out=wvb, in_=wvb_f)
        bk_col = singles.tile([D, 1], F32)
        nc.sync.dma_start(out=bk_col, in_=bk_h[:].rearrange("(d a) -> d a", a=1))
        bv_f = singles.tile([1, D], F32)
        nc.sync.dma_start(out=bv_f, in_=bv_h[:].rearrange("(a d) -> a d", a=1))
        bv_r = singles.tile([1, D], BF16)
        nc.vector.tensor_copy(out=bv_r, in_=bv_f)
        ones_f = singles.tile([1, 128], F32)
        nc.vector.memset(ones_f, 1.0)
        ones_r = singles.tile([1, 128], BF16)
        nc.vector.tensor_copy(out=ones_r, in_=ones_f)

        # Wq chunks [128, 2, D], bq as per-partition column [D, 1]
        wq_f = singles.tile([128, S_IN // 128, D], F32)
        nc.sync.dma_start(out=wq_f, in_=wq_h[:, :].rearrange("(c p) d -> p c d", p=128))
        wq_s = singles.tile([128, S_IN // 128, D], BF16)
        nc.vector.tensor_copy(out=wq_s, in_=wq_f)
        bq_s = singles.tile([D, 1], F32)
        nc.sync.dma_start(out=bq_s, in_=bq_h[:].rearrange("(d a) -> d a", a=1))

        # YK^T = [Wk;bk]^T @ yTe : [D, ny]
        ykt = singles.tile([D, ny], BF16)
        for b in range(ny // 512):
            yk_ps = psum_m.tile([D, 512], F32, tag="m")
            nc.tensor.matmul(
                yk_ps, (wkb), (yTe[:, b * 512:(b + 1) * 512]),
                start=True, stop=True,
            )
            nc.vector.tensor_scalar_add(out=ykt[:, b * 512:(b + 1) * 512],
                                        in0=yk_ps, scalar1=bk_col)

        # YV chunks with ones column: yve [128, nchunks, D+1]
        yve = singles.tile([128, nchunks, D + 1], BF16)
        assert nchunks % 4 == 0
        for g in range(nchunks // 4):
            yv_ps = psum_m.tile([128, 4 * D], F32, tag="m")
            for k in range(4):
                c = g * 4 + k
                nc.tensor.matmul(
                    yv_ps[:, k * D:(k + 1) * D],
                    (yTe[:, c * 128:(c + 1) * 128]), (wvb),
                    start=True, stop=False,
                )
                nc.tensor.matmul(
                    yv_ps[:, k * D:(k + 1) * D], ones_r, bv_r,
                    start=False, stop=True,
                )
            nc.vector.tensor_copy(
                out=yve[:, g * 4:(g + 1) * 4, 0:D],
                in_=yv_ps.rearrange("p (k d) -> p k d", k=4),
            )
        ones_ch = singles.tile([128, nchunks], F32)
        nc.vector.memset(ones_ch, 1.0)
        nc.vector.tensor_copy(out=yve[:, :, D:D + 1], in_=ones_ch)

        # ------------------------------------------------------------------
        # Main loop over row tiles
        # ------------------------------------------------------------------
        for (r0, R) in tiles:
            C = R // 128  # row sub-chunks

            # load x rows [R, S_IN] -> [128, C, S_IN]
            xs_t = xin_pool.tile([128, C, S_IN], F32, tag="xin")
            nc.sync.dma_start(
                out=xs_t,
                in_=xs_h[r0:r0 + R, :].rearrange("(s p) k -> p s k", p=128),
            )

            # transpose to x^T: two k-chunks [128, R]
            xt_ps = psum_m.tile([128, 2 * R], F32, tag="m")
            for s in range(C):
                for c in range(2):
                    nc.tensor.transpose(
                        xt_ps[:, c * R + s * 128: c * R + (s + 1) * 128],
                        xs_t[:, s, c * 128:(c + 1) * 128],
                        ident,
                    )
            xT = xt_pool.tile([128, 2, R], BF16, tag="xt")
            nc.scalar.activation(out=xT.rearrange("p a b -> p (a b)"), in_=xt_ps,
                                 func=AF.Copy)

            # XQ^T [D, R] = Wq^T @ x^T + bq
            xq_ps = psum_m.tile([D, R], F32, tag="m")
            nc.tensor.matmul(xq_ps, (wq_s[:, 0, :]), (xT[:, 0, :]),
                             start=True, stop=False)
            nc.tensor.matmul(xq_ps, (wq_s[:, 1, :]), (xT[:, 1, :]),
                             start=False, stop=True)
            xqt = xq_pool.tile([D, R], BF16, tag="xq")
            nc.vector.tensor_scalar_add(out=xqt, in0=xq_ps, scalar1=bq_s)

            # P accumulators [D+1, R]; row D collects S (from e) / R (from relu)
            p2_ps = psum_acc.tile([D + 1, R], F32, tag="p2")
            p1_ps = psum_acc.tile([D + 1, R], F32, tag="p1")

            per_group = GROUP_W // R
            for g0 in range(0, nchunks, per_group):
                grp = list(range(g0, min(g0 + per_group, nchunks)))
                W = len(grp) * R
                mt = psum_m.tile([128, W], F32, tag="m")
                for k, j in enumerate(grp):
                    nc.tensor.matmul(
                        mt[:, k * R:(k + 1) * R],
                        (ykt[:, j * 128:(j + 1) * 128]), (xqt),
                        start=True, stop=True,
                    )
                eg = er_pool.tile([128, GROUP_W], BF16, tag="e")
                rg = er_pool.tile([128, GROUP_W], BF16, tag="r")
                nc.scalar.activation(out=eg[:, 0:W], in_=mt, func=AF.Exp)
                if g0 == 0:
                    # balance: ACT does one relu group per tile (same table)
                    nc.scalar.activation(out=rg[:, 0:W], in_=mt, func=AF.Relu)
                else:
                    nc.vector.tensor_scalar_max(out=rg[:, 0:W], in0=mt, scalar1=0.0)
                for k, j in enumerate(grp):
                    st, sp = (j == 0), (j == nchunks - 1)
                    nc.tensor.matmul(p2_ps, (yve[:, j, :]),
                                     (eg[:, k * R:(k + 1) * R]),
                                     start=st, stop=sp)
                    nc.tensor.matmul(p1_ps, (yve[:, j, :]),
                                     (rg[:, k * R:(k + 1) * R]),
                                     start=st, stop=sp)

            # ---------------- tail: combine and store ----------------
            p2_s = tail_pool.tile([D + 1, R], F32, tag="p2s")
            p1_s = tail_pool.tile([D + 1, R], F32, tag="p1s")
            nc.vector.tensor_copy(out=p2_s, in_=p2_ps)
            nc.vector.tensor_copy(out=p1_s, in_=p1_ps)

            # S and R rows moved to partition base 0, then transposed to
            # per-row columns st/rt [128, C]
            sr = tail_pool.tile([1, 2 * R], F32, tag="sr")
            nc.vector.tensor_copy(out=sr[0:1, 0:R], in_=p2_s[D:D + 1, :])
            nc.vector.tensor_copy(out=sr[0:1, R:2 * R], in_=p1_s[D:D + 1, :])

            srt_ps = psum_m.tile([128, 2 * C], F32, tag="m")
            for rc in range(C):
                nc.tensor.transpose(
                    srt_ps[:, rc:rc + 1], sr[0:1, rc * 128:(rc + 1) * 128],
                    ident[0:1, 0:1],
                )
                nc.tensor.transpose(
                    srt_ps[:, C + rc:C + rc + 1],
                    sr[0:1, R + rc * 128:R + (rc + 1) * 128],
                    ident[0:1, 0:1],
                )
            # a = 0.1/den, b = 1/(S*den), den = 1 + 0.1*R
            den = tail_pool.tile([128, C], F32, tag="den")
            nc.vector.tensor_scalar(out=den, in0=srt_ps[:, C:2 * C],
                                    scalar1=0.1, scalar2=1.0,
                                    op0=ALU.mult, op1=ALU.add)
            a_t = tail_pool.tile([128, C], F32, tag="a")
            nc.vector.reciprocal(out=a_t, in_=den)
            nc.vector.tensor_scalar_mul(out=a_t, in0=a_t, scalar1=0.1)
            b_t = tail_pool.tile([128, C], F32, tag="b")
            nc.vector.tensor_mul(out=b_t, in0=srt_ps[:, 0:C], in1=den)
            nc.vector.reciprocal(out=b_t, in_=b_t)

            # transpose P1/P2 back to [128 rows, D] chunks
            o_ps = psum_m.tile([128, 2 * C * D], F32, tag="m")
            for rc in range(C):
                nc.tensor.transpose(
                    o_ps[:, rc * D:(rc + 1) * D],
                    p1_s[0:D, rc * 128:(rc + 1) * 128], ident[0:D, 0:D],
                )
                nc.tensor.transpose(
                    o_ps[:, (C + rc) * D:(C + rc + 1) * D],
                    p2_s[0:D, rc * 128:(rc + 1) * 128], ident[0:D, 0:D],
                )

            ot = tail_pool.tile([128, C, D], F32, tag="ot")
            t1 = tail_pool.tile([128, D], F32, tag="t1")
            t2 = tail_pool.tile([128, D], F32, tag="t2")
            for rc in range(C):
                nc.vector.tensor_scalar_mul(
                    out=t1, in0=o_ps[:, rc * D:(rc + 1) * D],
                    scalar1=a_t[:, rc:rc + 1])
                nc.vector.tensor_scalar_mul(
                    out=t2, in0=o_ps[:, (C + rc) * D:(C + rc + 1) * D],
                    scalar1=b_t[:, rc:rc + 1])
                nc.vector.tensor_add(out=ot[:, rc, :], in0=t1, in1=t2)

            nc.sync.dma_start(
                out=out_h[r0:r0 + R, :].rearrange("(s p) d -> p s d", p=128),
                in_=ot,
            )

    if split_waits:
        _split_multi_waits(nc)
    return nc


_NC_CACHE = {}

# test-harness knobs (the grading harness uses the defaults)
TRACE = False
LAST_RESULT = None


def _get_nc(pad_rows, ny):
    key = (pad_rows, ny)
    if key not in _NC_CACHE:
        _NC_CACHE[key] = build_nc(pad_rows, ny)
    return _NC_CACHE[key]


def kernel(x, y, Wq, bq, Wk, bk, Wv, bv):
    from concourse.bass_utils import run_bass_kernel_spmd

    x = np.ascontiguousarray(np.asarray(x, dtype=np.float32))
    y = np.ascontiguousarray(np.asarray(y, dtype=np.float32))
    Wq = np.asarray(Wq, np.float32)
    bq = np.asarray(bq, np.float32)
    Wk = np.asarray(Wk, np.float32)
    bk = np.asarray(bk, np.float32)
    Wv = np.asarray(Wv, np.float32)
    bv = np.asarray(bv, np.float32)

    n = x.shape[0]
    rows_per_core = (n + N_CORES - 1) // N_CORES
    pad_rows = ((rows_per_core + 127) // 128) * 128
    ny = y.shape[0]

    nc = _get_nc(pad_rows, ny)

    xp = np.zeros((N_CORES, pad_rows, S_IN), np.float32)
    for c in range(N_CORES):
        lo = c * rows_per_core
        hi = min(lo + rows_per_core, n)
        xp[c, 0:hi - lo] = x[lo:hi]

    common = {"y": y, "Wq": Wq, "bq": bq, "Wk": Wk, "bk": bk, "Wv": Wv, "bv": bv}
    in_maps = [{"xs": xp[c], **common} for c in range(N_CORES)]
    kwargs = {}
    if TRACE:
        import shutil, tempfile
        shutil.rmtree("/tmp/kern_trace", ignore_errors=True)
        kwargs = dict(trace=True, tmpdir="/tmp/kern_trace")
    res = run_bass_kernel_spmd(nc, in_maps, core_ids=list(range(N_CORES)), **kwargs)
    global LAST_RESULT
    LAST_RESULT = res

    out = np.empty((n, D), np.float32)
    for c in range(N_CORES):
        lo = c * rows_per_core
        hi = min(lo + rows_per_core, n)
        out[lo:hi] = res.results[c]["out"][0:hi - lo]
    return out


# revision 19
# speedup vs baseline: 1.1331x; 1.1331x over previous
"""Trainium2 Bass kernel for nn_AttentionBlock (smooth-softmax attention).

  out = smoothsoftmax((x@Wq+bq) @ (y@Wk+bk)^T) @ (y@Wv+bv)
  smoothsoftmax(M) = (0.1*relu(M) + softmax(M)) / rowsum(...)

Strategy (per core, x row-sharded across 8 cores):
  - Everything kept transposed on-chip: M^T chunks [128 keys, R rows] so both
    the softmax denominator and the output contraction over the 4096 key dim
    become PE matmul accumulations (no large transposes, no cross-partition
    reductions).
  - exp without max-subtraction (|M| <= ~20 so fp32 exp is safe).
  - out_i = (0.1*P1_i + P2_i/S_i) / (1 + 0.1*R_i) with
      P1 = relu(M) @ YV, P2 = e^M @ YV, S = rowsum(e^M), R = rowsum(relu(M));
    S and R come free as a ones-column appended to YV.
"""

import numpy as np
from contextlib import ExitStack

import concourse.bass as bass
import concourse.mybir as mybir
import concourse.tile as tile
from concourse.masks import make_identity

# ----------------------------------------------------------------------------
# Workaround for walrus "Too many sync wait commands" on the TileContext
# kernel-tail Drain: pre-issue the global-clock waits on the sync engine one
# per nop before the drain; the drain itself then needs no waits (SP executes
# in order).
from concourse.vector_clock import ScopedClock, VectorClock


def _drain_and_barrier_split(self, tick_clock, wait_clock):
    gc = tick_clock.global_clock
    n = len(gc)
    procs = [p for p in range(n) if gc[p] > 0]
    for p in procs:
        vec = [gc[q] if q == p else 0 for q in range(n)]
        nop = self.nc.sync.nop(nofuse=True, hint="drain_wait_split")
        wait_clock.add_sem_waits(nop.ins, ScopedClock({None: VectorClock(vec)}))
    self.nc.sync.drain()
    self.nc.all_engine_barrier()
    assert self.sems is not None
    popped = self.nc._tile_sem_poison_stack.pop()
    assert popped is self._sem_poison
    self.nc.clear_and_free_semaphores(list(self.sems.allocated().values()))
    self.nc.all_engine_barrier()


tile.TileContext._drain_and_barrier = _drain_and_barrier_split

from concourse import bass_utils as _bu

_orig_run_command = _bu.run_command


def _run_command_ldwopt(argv, **kwargs):
    argv = list(argv)
    return _orig_run_command(argv, **kwargs)


_bu.run_command = _run_command_ldwopt


def _split_multi_waits(nc, max_waits=1):
    """This walrus build rejects instructions carrying more than one sync
    wait.  Hoist extra waits onto single-wait NoOps on the same engine
    immediately before the instruction (engine streams execute in order,
    so semantics are identical)."""
    for f in nc.m.functions:
        for b in f.blocks:
            out = []
            changed = False
            for inst in b.instructions:
                si = inst.sync_info
                if si is not None and si.on_wait and len(si.on_wait) > max_waits:
                    waits = list(si.on_wait)
                    for w in waits[max_waits:]:
                        out.append(mybir.InstNoOp(
                            name=nc.get_next_instruction_name(),
                            engine=inst.engine,
                            bass_nofuse=True,
                            sync_info=mybir.SyncInfo(on_wait=[w], on_update=[]),
                        ))
                    si.on_wait = waits[:max_waits]
                    changed = True
                out.append(inst)
            if changed:
                b.instructions = out
# ----------------------------------------------------------------------------

F32 = mybir.dt.float32
F32R = mybir.dt.float32r
BF16 = mybir.dt.bfloat16

N_CORES = 8
N_FULL = 50000
S_IN = 256
NY = 4096
YDIM = 7
D = 64

ROWS_PER_CORE = (N_FULL + N_CORES - 1) // N_CORES  # 6250
PAD_ROWS = ((ROWS_PER_CORE + 127) // 128) * 128    # 6272

AF = mybir.ActivationFunctionType
ALU = mybir.AluOpType


def build_nc(pad_rows=PAD_ROWS, ny=NY, big_tile=512, split_waits=True):
    """Build the per-core Bass program. All 8 cores run the same program on
    different x shards (y and the projection weights are replicated)."""
    nc = bass.Bass(trn_type="TRN2")

    xs_h = nc.dram_tensor("xs", [pad_rows, S_IN], F32, kind="ExternalInput")
    y_h = nc.dram_tensor("y", [ny, YDIM], F32, kind="ExternalInput")
    wq_h = nc.dram_tensor("Wq", [S_IN, D], F32, kind="ExternalInput")
    bq_h = nc.dram_tensor("bq", [D], F32, kind="ExternalInput")
    wk_h = nc.dram_tensor("Wk", [YDIM, D], F32, kind="ExternalInput")
    bk_h = nc.dram_tensor("bk", [D], F32, kind="ExternalInput")
    wv_h = nc.dram_tensor("Wv", [YDIM, D], F32, kind="ExternalInput")
    bv_h = nc.dram_tensor("bv", [D], F32, kind="ExternalInput")
    out_h = nc.dram_tensor("out", [pad_rows, D], F32, kind="ExternalOutput")

    nchunks = ny // 128          # key chunks of 128
    assert ny % 512 == 0

    # row tiles: big_tile-row tiles then a 128-multiple remainder tile
    tiles = []
    r0 = 0
    while r0 + big_tile <= pad_rows:
        tiles.append((r0, big_tile))
        r0 += big_tile
    if r0 < pad_rows:
        assert (pad_rows - r0) % 128 == 0
        tiles.append((r0, pad_rows - r0))

    GROUP_W = 1024  # fp32 elems per partition in one m-psum slot (2 banks)

    with tile.TileContext(nc) as tc, ExitStack() as ctx:
        singles = ctx.enter_context(tc.tile_pool(name="singles", bufs=1))
        psum_m = ctx.enter_context(tc.tile_pool(name="psum_m", bufs=3, space="PSUM"))
        psum_acc = ctx.enter_context(tc.tile_pool(name="psum_acc", bufs=1, space="PSUM"))
        xin_pool = ctx.enter_context(tc.tile_pool(name="xin", bufs=2))
        xt_pool = ctx.enter_context(tc.tile_pool(name="xt", bufs=2))
        xq_pool = ctx.enter_context(tc.tile_pool(name="xq", bufs=2))
        er_pool = ctx.enter_context(tc.tile_pool(name="er", bufs=4))
        tail_pool = ctx.enter_context(tc.tile_pool(name="tail", bufs=2))

        # ------------------------------------------------------------------
        # Constants / precompute (once per core)
        # ------------------------------------------------------------------
        ident = singles.tile([128, 128], F32)
        make_identity(nc, ident)

        # y^T with a ones row appended: yTe [8, ny]
        ys = singles.tile([128, nchunks, YDIM], F32)
        nc.sync.dma_start(out=ys, in_=y_h[:, :].rearrange("(c p) d -> p c d", p=128))
        yTe = singles.tile([YDIM, ny], BF16)
        assert nchunks % 4 == 0
        for g in range(nchunks // 4):
            yt_ps = psum_m.tile([8, 512], F32, tag="m")
            for k in range(4):
                c = g * 4 + k
                nc.tensor.transpose(
                    yt_ps[0:YDIM, k * 128:(k + 1) * 128], ys[:, c, :], ident
                )
            nc.vector.tensor_copy(
                out=yTe[0:YDIM, g * 512:(g + 1) * 512], in_=yt_ps[0:YDIM, :]
            )

        # [Wk; bk] and [Wv; bv] stacked: [8, D]
        wkb_f = singles.tile([YDIM, D], F32)
        nc.sync.dma_start(out=wkb_f, in_=wk_h[:, :])
        wkb = singles.tile([YDIM, D], BF16)
        nc.vector.tensor_copy(out=wkb, in_=wkb_f)
        wvb_f = singles.tile([YDIM, D], F32)
        nc.sync.dma_start(out=wvb_f, in_=wv_h[:, :])
        wvb = singles.tile([YDIM, D], BF16)
        nc.vector.tensor_copy(out=wvb, in_=wvb_f)
        bk_col = singles.tile([D, 1], F32)
        nc.sync.dma_start(out=bk_col, in_=bk_h[:].rearrange("(d a) -> d a", a=1))
        bv_f = singles.tile([1, D], F32)
        nc.sync.dma_start(out=bv_f, in_=bv_h[:].rearrange("(a d) -> a d", a=1))
        bv_r = singles.tile([1, D], BF16)
        nc.vector.tensor_copy(out=bv_r, in_=bv_f)
        ones_f = singles.tile([1, 128], F32)
        nc.vector.memset(ones_f, 1.0)
        ones_r = singles.tile([1, 128], BF16)
        nc.vector.tensor_copy(out=ones_r, in_=ones_f)

        # Wq chunks [128, 2, D], bq as per-partition column [D, 1]
        wq_f = singles.tile([128, S_IN // 128, D], F32)
        nc.sync.dma_start(out=wq_f, in_=wq_h[:, :].rearrange("(c p) d -> p c d", p=128))
        wq_s = singles.tile([128, S_IN // 128, D], BF16)
        nc.vector.tensor_copy(out=wq_s, in_=wq_f)
        bq_s = singles.tile([D, 1], F32)
        nc.sync.dma_start(out=bq_s, in_=bq_h[:].rearrange("(d a) -> d a", a=1))

        # YK^T = [Wk;bk]^T @ yTe : [D, ny]
        ykt = singles.tile([D, ny], BF16)
        for b in range(ny // 512):
            yk_ps = psum_m.tile([D, 512], F32, tag="m")
            nc.tensor.matmul(
                yk_ps, (wkb), (yTe[:, b * 512:(b + 1) * 512]),
                start=True, stop=True,
            )
            nc.vector.tensor_scalar_add(out=ykt[:, b * 512:(b + 1) * 512],
                                        in0=yk_ps, scalar1=bk_col)

        # YV chunks with ones column: yve [128, nchunks, D+1]
        yve = singles.tile([128, nchunks, D + 1], BF16)
        assert nchunks % 4 == 0
        for g in range(nchunks // 4):
            yv_ps = psum_m.tile([128, 4 * D], F32, tag="m")
            for k in range(4):
                c = g * 4 + k
                nc.tensor.matmul(
                    yv_ps[:, k * D:(k + 1) * D],
                    (yTe[:, c * 128:(c + 1) * 128]), (wvb),
                    start=True, stop=False,
                )
                nc.tensor.matmul(
                    yv_ps[:, k * D:(k + 1) * D], ones_r, bv_r,
                    start=False, stop=True,
                )
            nc.vector.tensor_copy(
                out=yve[:, g * 4:(g + 1) * 4, 0:D],
                in_=yv_ps.rearrange("p (k d) -> p k d", k=4),
            )
        ones_ch = singles.tile([128, nchunks], F32)
        nc.vector.memset(ones_ch, 1.0)
        nc.vector.tensor_copy(out=yve[:, :, D:D + 1], in_=ones_ch)

        # ------------------------------------------------------------------
        # Main loop over row tiles
        # ------------------------------------------------------------------
        def emit_head(r0, R):
            """Load + transpose + project one row tile; returns XQ^T [D, R]."""
            C = R // 128
            xs_t = xin_pool.tile([128, C, S_IN], F32, tag="xin")
            nc.sync.dma_start(
                out=xs_t,
                in_=xs_h[r0:r0 + R, :].rearrange("(s p) k -> p s k", p=128),
            )
            xt_ps = psum_m.tile([128, 2 * R], F32, tag="m")
            for s in range(C):
                for c in range(2):
                    nc.tensor.transpose(
                        xt_ps[:, c * R + s * 128: c * R + (s + 1) * 128],
                        xs_t[:, s, c * 128:(c + 1) * 128],
                        ident,
                    )
            xT = xt_pool.tile([128, 2, R], BF16, tag="xt")
            nc.scalar.activation(out=xT.rearrange("p a b -> p (a b)"), in_=xt_ps,
                                 func=AF.Copy)
            xq_ps = psum_m.tile([D, R], F32, tag="m")
            nc.tensor.matmul(xq_ps, (wq_s[:, 0, :]), (xT[:, 0, :]),
                             start=True, stop=False)
            nc.tensor.matmul(xq_ps, (wq_s[:, 1, :]), (xT[:, 1, :]),
                             start=False, stop=True)
            xqt = xq_pool.tile([D, R], BF16, tag="xq")
            nc.vector.tensor_scalar_add(out=xqt, in0=xq_ps, scalar1=bq_s)
            return xqt

        next_xqt = emit_head(*tiles[0])
        for ti, (r0, R) in enumerate(tiles):
            C = R // 128  # row sub-chunks
            xqt = next_xqt

            # P accumulators [D+1, R]; row D collects S (from e) / R (from relu)
            p2_ps = psum_acc.tile([D + 1, R], F32, tag="p2")
            p1_ps = psum_acc.tile([D + 1, R], F32, tag="p1")

            per_group = GROUP_W // R
            groups = list(range(0, nchunks, per_group))
            for gi, g0 in enumerate(groups):
                grp = list(range(g0, min(g0 + per_group, nchunks)))
                W = len(grp) * R
                mt = psum_m.tile([128, W], F32, tag="m")
                for k, j in enumerate(grp):
                    nc.tensor.matmul(
                        mt[:, k * R:(k + 1) * R],
                        (ykt[:, j * 128:(j + 1) * 128]), (xqt),
                        start=True, stop=True,
                    )
                eg = er_pool.tile([128, GROUP_W], BF16, tag="e")
                rg = er_pool.tile([128, GROUP_W], BF16, tag="r")
                nc.scalar.activation(out=eg[:, 0:W], in_=mt, func=AF.Exp)
                if g0 == 0:
                    # balance: ACT does one relu group per tile (same table)
                    nc.scalar.activation(out=rg[:, 0:W], in_=mt, func=AF.Relu)
                else:
                    nc.vector.tensor_scalar_max(out=rg[:, 0:W], in0=mt, scalar1=0.0)
                for k, j in enumerate(grp):
                    st, sp = (j == 0), (j == nchunks - 1)
                    nc.tensor.matmul(p2_ps, (yve[:, j, :]),
                                     (eg[:, k * R:(k + 1) * R]),
                                     start=st, stop=sp)
                    nc.tensor.matmul(p1_ps, (yve[:, j, :]),
                                     (rg[:, k * R:(k + 1) * R]),
                                     start=st, stop=sp)
                if gi == len(groups) // 2 and ti + 1 < len(tiles):
                    # software pipeline: emit next tile's head mid-loop so the
                    # scheduler overlaps it with this tile's group stream
                    next_xqt = emit_head(*tiles[ti + 1])

            # ---------------- tail: combine and store ----------------
            p2_s = tail_pool.tile([D + 1, R], F32, tag="p2s")
            p1_s = tail_pool.tile([D + 1, R], F32, tag="p1s")
            nc.vector.tensor_copy(out=p2_s, in_=p2_ps)
            nc.vector.tensor_copy(out=p1_s, in_=p1_ps)

            # S and R rows moved to partition base 0, then transposed to
            # per-row columns st/rt [128, C]
            sr = tail_pool.tile([1, 2 * R], F32, tag="sr")
            nc.vector.tensor_copy(out=sr[0:1, 0:R], in_=p2_s[D:D + 1, :])
            nc.vector.tensor_copy(out=sr[0:1, R:2 * R], in_=p1_s[D:D + 1, :])

            srt_ps = psum_m.tile([128, 2 * C], F32, tag="m")
            for rc in range(C):
                nc.tensor.transpose(
                    srt_ps[:, rc:rc + 1], sr[0:1, rc * 128:(rc + 1) * 128],
                    ident[0:1, 0:1],
                )
                nc.tensor.transpose(
                    srt_ps[:, C + rc:C + rc + 1],
                    sr[0:1, R + rc * 128:R + (rc + 1) * 128],
                    ident[0:1, 0:1],
                )
            # a = 0.1/den, b = 1/(S*den), den = 1 + 0.1*R
            den = tail_pool.tile([128, C], F32, tag="den")
            nc.vector.tensor_scalar(out=den, in0=srt_ps[:, C:2 * C],
                                    scalar1=0.1, scalar2=1.0,
                                    op0=ALU.mult, op1=ALU.add)
            a_t = tail_pool.tile([128, C], F32, tag="a")
            nc.vector.reciprocal(out=a_t, in_=den)
            nc.vector.tensor_scalar_mul(out=a_t, in0=a_t, scalar1=0.1)
            b_t = tail_pool.tile([128, C], F32, tag="b")
            nc.vector.tensor_mul(out=b_t, in0=srt_ps[:, 0:C], in1=den)
            nc.vector.reciprocal(out=b_t, in_=b_t)

            # transpose P1/P2 back to [128 rows, D] chunks
            o_ps = psum_m.tile([128, 2 * C * D], F32, tag="m")
            for rc in range(C):
                nc.tensor.transpose(
                    o_ps[:, rc * D:(rc + 1) * D],
                    p1_s[0:D, rc * 128:(rc + 1) * 128], ident[0:D, 0:D],
                )
                nc.tensor.transpose(
                    o_ps[:, (C + rc) * D:(C + rc + 1) * D],
                    p2_s[0:D, rc * 128:(rc + 1) * 128], ident[0:D, 0:D],
                )

            ot = tail_pool.tile([128, C, D], F32, tag="ot")
            t1 = tail_pool.tile([128, D], F32, tag="t1")
            t2 = tail_pool.tile([128, D], F32, tag="t2")
            for rc in range(C):
                nc.vector.tensor_scalar_mul(
                    out=t1, in0=o_ps[:, rc * D:(rc + 1) * D],
                    scalar1=a_t[:, rc:rc + 1])
                nc.vector.tensor_scalar_mul(
                    out=t2, in0=o_ps[:, (C + rc) * D:(C + rc + 1) * D],
                    scalar1=b_t[:, rc:rc + 1])
                nc.vector.tensor_add(out=ot[:, rc, :], in0=t1, in1=t2)

            nc.sync.dma_start(
                out=out_h[r0:r0 + R, :].rearrange("(s p) d -> p s d", p=128),
                in_=ot,
            )

    if split_waits:
        _split_multi_waits(nc)
    return nc


_NC_CACHE = {}

# test-harness knobs (the grading harness uses the defaults)
TRACE = False
LAST_RESULT = None


def _get_nc(pad_rows, ny):
    key = (pad_rows, ny)
    if key not in _NC_CACHE:
        _NC_CACHE[key] = build_nc(pad_rows, ny)
    return _NC_CACHE[key]


def kernel(x, y, Wq, bq, Wk, bk, Wv, bv):
    from concourse.bass_utils import run_bass_kernel_spmd

    x = np.ascontiguousarray(np.asarray(x, dtype=np.float32))
    y = np.ascontiguousarray(np.asarray(y, dtype=np.float32))
    Wq = np.asarray(Wq, np.float32)
    bq = np.asarray(bq, np.float32)
    Wk = np.asarray(Wk, np.float32)
    bk = np.asarray(bk, np.float32)
    Wv = np.asarray(Wv, np.float32)
    bv = np.asarray(bv, np.float32)

    n = x.shape[0]
    rows_per_core = (n + N_CORES - 1) // N_CORES
    pad_rows = ((rows_per_core + 127) // 128) * 128
    ny = y.shape[0]

    nc = _get_nc(pad_rows, ny)

    xp = np.zeros((N_CORES, pad_rows, S_IN), np.float32)
    for c in range(N_CORES):
        lo = c * rows_per_core
        hi = min(lo + rows_per_core, n)
        xp[c, 0:hi - lo] = x[lo:hi]

    common = {"y": y, "Wq": Wq, "bq": bq, "Wk": Wk, "bk": bk, "Wv": Wv, "bv": bv}
    in_maps = [{"xs": xp[c], **common} for c in range(N_CORES)]
    kwargs = {}
    if TRACE:
        import shutil, tempfile
        shutil.rmtree("/tmp/kern_trace", ignore_errors=True)
        kwargs = dict(trace=True, tmpdir="/tmp/kern_trace")
    res = run_bass_kernel_spmd(nc, in_maps, core_ids=list(range(N_CORES)), **kwargs)
    global LAST_RESULT
    LAST_RESULT = res

    out = np.empty((n, D), np.float32)
    for c in range(N_CORES):
        lo = c * rows_per_core
        hi = min(lo + rows_per_core, n)
        out[lo:hi] = res.results[c]["out"][0:hi - lo]
    return out


# revision 20
# speedup vs baseline: 1.2469x; 1.1004x over previous
"""Trainium2 Bass kernel for nn_AttentionBlock (smooth-softmax attention).

  out = smoothsoftmax((x@Wq+bq) @ (y@Wk+bk)^T) @ (y@Wv+bv)
  smoothsoftmax(M) = (0.1*relu(M) + softmax(M)) / rowsum(...)

Strategy (per core, x row-sharded across 8 cores):
  - Everything kept transposed on-chip: M^T chunks [128 keys, R rows] so both
    the softmax denominator and the output contraction over the 4096 key dim
    become PE matmul accumulations (no large transposes, no cross-partition
    reductions).
  - exp without max-subtraction (|M| <= ~20 so fp32 exp is safe).
  - out_i = (0.1*P1_i + P2_i/S_i) / (1 + 0.1*R_i) with
      P1 = relu(M) @ YV, P2 = e^M @ YV, S = rowsum(e^M), R = rowsum(relu(M));
    S and R come free as a ones-column appended to YV.
"""

import numpy as np
from contextlib import ExitStack

import concourse.bass as bass
import concourse.mybir as mybir
import concourse.tile as tile
from concourse.masks import make_identity

# ----------------------------------------------------------------------------
# Workaround for walrus "Too many sync wait commands" on the TileContext
# kernel-tail Drain: pre-issue the global-clock waits on the sync engine one
# per nop before the drain; the drain itself then needs no waits (SP executes
# in order).
from concourse.vector_clock import ScopedClock, VectorClock


def _drain_and_barrier_split(self, tick_clock, wait_clock):
    gc = tick_clock.global_clock
    n = len(gc)
    procs = [p for p in range(n) if gc[p] > 0]
    for p in procs:
        vec = [gc[q] if q == p else 0 for q in range(n)]
        nop = self.nc.sync.nop(nofuse=True, hint="drain_wait_split")
        wait_clock.add_sem_waits(nop.ins, ScopedClock({None: VectorClock(vec)}))
    self.nc.sync.drain()
    self.nc.all_engine_barrier()
    assert self.sems is not None
    popped = self.nc._tile_sem_poison_stack.pop()
    assert popped is self._sem_poison
    self.nc.clear_and_free_semaphores(list(self.sems.allocated().values()))
    self.nc.all_engine_barrier()


tile.TileContext._drain_and_barrier = _drain_and_barrier_split

from concourse import bass_utils as _bu

_orig_run_command = _bu.run_command


def _run_command_ldwopt(argv, **kwargs):
    argv = list(argv)
    return _orig_run_command(argv, **kwargs)


_bu.run_command = _run_command_ldwopt


def _split_multi_waits(nc, max_waits=1):
    """This walrus build rejects instructions carrying more than one sync
    wait.  Hoist extra waits onto single-wait NoOps on the same engine
    immediately before the instruction (engine streams execute in order,
    so semantics are identical)."""
    for f in nc.m.functions:
        for b in f.blocks:
            out = []
            changed = False
            for inst in b.instructions:
                si = inst.sync_info
                if si is not None and si.on_wait and len(si.on_wait) > max_waits:
                    waits = list(si.on_wait)
                    for w in waits[max_waits:]:
                        out.append(mybir.InstNoOp(
                            name=nc.get_next_instruction_name(),
                            engine=inst.engine,
                            bass_nofuse=True,
                            sync_info=mybir.SyncInfo(on_wait=[w], on_update=[]),
                        ))
                    si.on_wait = waits[:max_waits]
                    changed = True
                out.append(inst)
            if changed:
                b.instructions = out
# ----------------------------------------------------------------------------

F32 = mybir.dt.float32
F32R = mybir.dt.float32r
BF16 = mybir.dt.bfloat16

N_CORES = 8
N_FULL = 50000
S_IN = 256
NY = 4096
YDIM = 7
D = 64

ROWS_PER_CORE = (N_FULL + N_CORES - 1) // N_CORES  # 6250
PAD_ROWS = ((ROWS_PER_CORE + 127) // 128) * 128    # 6272

AF = mybir.ActivationFunctionType
ALU = mybir.AluOpType


def build_nc(pad_rows=PAD_ROWS, ny=NY, big_tile=512, split_waits=True):
    """Build the per-core Bass program. All 8 cores run the same program on
    different x shards (y and the projection weights are replicated)."""
    nc = bass.Bass(trn_type="TRN2")

    xs_h = nc.dram_tensor("xs", [pad_rows, S_IN], F32, kind="ExternalInput")
    y_h = nc.dram_tensor("y", [ny, YDIM], F32, kind="ExternalInput")
    wq_h = nc.dram_tensor("Wq", [S_IN, D], F32, kind="ExternalInput")
    bq_h = nc.dram_tensor("bq", [D], F32, kind="ExternalInput")
    wk_h = nc.dram_tensor("Wk", [YDIM, D], F32, kind="ExternalInput")
    bk_h = nc.dram_tensor("bk", [D], F32, kind="ExternalInput")
    wv_h = nc.dram_tensor("Wv", [YDIM, D], F32, kind="ExternalInput")
    bv_h = nc.dram_tensor("bv", [D], F32, kind="ExternalInput")
    out_h = nc.dram_tensor("out", [pad_rows, D], F32, kind="ExternalOutput")

    nchunks = ny // 128          # key chunks of 128
    assert ny % 512 == 0

    # row tiles: big_tile-row tiles then a 128-multiple remainder tile
    tiles = []
    r0 = 0
    while r0 + big_tile <= pad_rows:
        tiles.append((r0, big_tile))
        r0 += big_tile
    if r0 < pad_rows:
        assert (pad_rows - r0) % 128 == 0
        tiles.append((r0, pad_rows - r0))

    GROUP_W = 1024  # fp32 elems per partition in one m-psum slot (2 banks)

    with tile.TileContext(nc) as tc, ExitStack() as ctx:
        singles = ctx.enter_context(tc.tile_pool(name="singles", bufs=1))
        psum_m = ctx.enter_context(tc.tile_pool(name="psum_m", bufs=3, space="PSUM"))
        psum_acc = ctx.enter_context(tc.tile_pool(name="psum_acc", bufs=1, space="PSUM"))
        xin_pool = ctx.enter_context(tc.tile_pool(name="xin", bufs=2))
        xt_pool = ctx.enter_context(tc.tile_pool(name="xt", bufs=2))
        xq_pool = ctx.enter_context(tc.tile_pool(name="xq", bufs=2))
        er_pool = ctx.enter_context(tc.tile_pool(name="er", bufs=4))
        tail_pool = ctx.enter_context(tc.tile_pool(name="tail", bufs=2))

        # ------------------------------------------------------------------
        # Constants / precompute (once per core)
        # ------------------------------------------------------------------
        ident = singles.tile([128, 128], F32)
        make_identity(nc, ident)

        # y^T with a ones row appended: yTe [8, ny]
        ys = singles.tile([128, nchunks, YDIM], F32)
        nc.sync.dma_start(out=ys, in_=y_h[:, :].rearrange("(c p) d -> p c d", p=128))
        yTe = singles.tile([YDIM, ny], BF16)
        assert nchunks % 4 == 0
        for g in range(nchunks // 4):
            yt_ps = psum_m.tile([8, 512], F32, tag="m")
            for k in range(4):
                c = g * 4 + k
                nc.tensor.transpose(
                    yt_ps[0:YDIM, k * 128:(k + 1) * 128], ys[:, c, :], ident
                )
            nc.vector.tensor_copy(
                out=yTe[0:YDIM, g * 512:(g + 1) * 512], in_=yt_ps[0:YDIM, :]
            )

        # [Wk; bk] and [Wv; bv] stacked: [8, D]
        wkb_f = singles.tile([YDIM, D], F32)
        nc.sync.dma_start(out=wkb_f, in_=wk_h[:, :])
        wkb = singles.tile([YDIM, D], BF16)
        nc.vector.tensor_copy(out=wkb, in_=wkb_f)
        wvb_f = singles.tile([YDIM, D], F32)
        nc.sync.dma_start(out=wvb_f, in_=wv_h[:, :])
        wvb = singles.tile([YDIM, D], BF16)
        nc.vector.tensor_copy(out=wvb, in_=wvb_f)
        bk_col = singles.tile([D, 1], F32)
        nc.sync.dma_start(out=bk_col, in_=bk_h[:].rearrange("(d a) -> d a", a=1))
        bv_f = singles.tile([1, D], F32)
        nc.sync.dma_start(out=bv_f, in_=bv_h[:].rearrange("(a d) -> a d", a=1))
        bv_r = singles.tile([1, D], BF16)
        nc.vector.tensor_copy(out=bv_r, in_=bv_f)
        ones_f = singles.tile([1, 128], F32)
        nc.vector.memset(ones_f, 1.0)
        ones_r = singles.tile([1, 128], BF16)
        nc.vector.tensor_copy(out=ones_r, in_=ones_f)

        # Wq chunks [128, 2, D], bq as per-partition column [D, 1]
        wq_f = singles.tile([128, S_IN // 128, D], F32)
        nc.sync.dma_start(out=wq_f, in_=wq_h[:, :].rearrange("(c p) d -> p c d", p=128))
        wq_s = singles.tile([128, S_IN // 128, D], BF16)
        nc.vector.tensor_copy(out=wq_s, in_=wq_f)
        bq_s = singles.tile([D, 1], F32)
        nc.sync.dma_start(out=bq_s, in_=bq_h[:].rearrange("(d a) -> d a", a=1))

        # YK^T = [Wk;bk]^T @ yTe : [D, ny]
        ykt = singles.tile([D, ny], BF16)
        for b in range(ny // 512):
            yk_ps = psum_m.tile([D, 512], F32, tag="m")
            nc.tensor.matmul(
                yk_ps, (wkb), (yTe[:, b * 512:(b + 1) * 512]),
                start=True, stop=True,
            )
            nc.vector.tensor_scalar_add(out=ykt[:, b * 512:(b + 1) * 512],
                                        in0=yk_ps, scalar1=bk_col)

        # YV chunks with ones column: yve [128, nchunks, D+1]
        yve = singles.tile([128, nchunks, D + 1], BF16)
        assert nchunks % 4 == 0
        for g in range(nchunks // 4):
            yv_ps = psum_m.tile([128, 4 * D], F32, tag="m")
            for k in range(4):
                c = g * 4 + k
                nc.tensor.matmul(
                    yv_ps[:, k * D:(k + 1) * D],
                    (yTe[:, c * 128:(c + 1) * 128]), (wvb),
                    start=True, stop=False,
                )
                nc.tensor.matmul(
                    yv_ps[:, k * D:(k + 1) * D], ones_r, bv_r,
                    start=False, stop=True,
                )
            nc.vector.tensor_copy(
                out=yve[:, g * 4:(g + 1) * 4, 0:D],
                in_=yv_ps.rearrange("p (k d) -> p k d", k=4),
            )
        ones_ch = singles.tile([128, nchunks], F32)
        nc.vector.memset(ones_ch, 1.0)
        nc.vector.tensor_copy(out=yve[:, :, D:D + 1], in_=ones_ch)

        # ------------------------------------------------------------------
        # Main loop over row tiles
        # ------------------------------------------------------------------
        def emit_head(r0, R):
            """Load + transpose + project one row tile; returns XQ^T [D, R]."""
            C = R // 128
            xs_t = xin_pool.tile([128, C, S_IN], F32, tag="xin")
            nc.sync.dma_start(
                out=xs_t,
                in_=xs_h[r0:r0 + R, :].rearrange("(s p) k -> p s k", p=128),
            )
            xt_ps = psum_m.tile([128, 2 * R], F32, tag="m")
            for s in range(C):
                for c in range(2):
                    nc.tensor.transpose(
                        xt_ps[:, c * R + s * 128: c * R + (s + 1) * 128],
                        xs_t[:, s, c * 128:(c + 1) * 128],
                        ident,
                    )
            xT = xt_pool.tile([128, 2, R], BF16, tag="xt")
            nc.scalar.activation(out=xT.rearrange("p a b -> p (a b)"), in_=xt_ps,
                                 func=AF.Copy)
            xq_ps = psum_m.tile([D, R], F32, tag="m")
            nc.tensor.matmul(xq_ps, (wq_s[:, 0, :]), (xT[:, 0, :]),
                             start=True, stop=False)
            nc.tensor.matmul(xq_ps, (wq_s[:, 1, :]), (xT[:, 1, :]),
                             start=False, stop=True)
            xqt = xq_pool.tile([D, R], BF16, tag="xq")
            nc.vector.tensor_scalar_add(out=xqt, in0=xq_ps, scalar1=bq_s)
            return xqt

        next_xqt = emit_head(*tiles[0])
        for ti, (r0, R) in enumerate(tiles):
            C = R // 128  # row sub-chunks
            xqt = next_xqt

            # P accumulators [D+1, R]; row D collects S (from e) / R (from relu)
            p2_ps = psum_acc.tile([D + 1, R], F32, tag="p2")
            p1_ps = psum_acc.tile([D + 1, R], F32, tag="p1")

            per_group = GROUP_W // R
            groups = list(range(0, nchunks, per_group))
            for gi, g0 in enumerate(groups):
                grp = list(range(g0, min(g0 + per_group, nchunks)))
                W = len(grp) * R
                mt = psum_m.tile([128, W], F32, tag="m")
                for k, j in enumerate(grp):
                    nc.tensor.matmul(
                        mt[:, k * R:(k + 1) * R],
                        (ykt[:, j * 128:(j + 1) * 128]), (xqt),
                        start=True, stop=True,
                    )
                eg = er_pool.tile([128, GROUP_W], BF16, tag="e")
                rg = er_pool.tile([128, GROUP_W], BF16, tag="r")
                nc.scalar.activation(out=eg[:, 0:W], in_=mt, func=AF.Exp)
                if g0 == 0:
                    # balance: ACT does one relu group per tile (same table)
                    nc.scalar.activation(out=rg[:, 0:W], in_=mt, func=AF.Relu)
                else:
                    nc.vector.tensor_scalar_max(out=rg[:, 0:W], in0=mt, scalar1=0.0)
                for k, j in enumerate(grp):
                    st, sp = (j == 0), (j == nchunks - 1)
                    nc.tensor.matmul(p2_ps, (yve[:, j, :]),
                                     (eg[:, k * R:(k + 1) * R]),
                                     start=st, stop=sp)
                    nc.tensor.matmul(p1_ps, (yve[:, j, :]),
                                     (rg[:, k * R:(k + 1) * R]),
                                     start=st, stop=sp)
                if gi == len(groups) // 2 and ti + 1 < len(tiles):
                    # software pipeline: emit next tile's head mid-loop so the
                    # scheduler overlaps it with this tile's group stream
                    next_xqt = emit_head(*tiles[ti + 1])

            # ---------------- tail: combine and store ----------------
            p2_s = tail_pool.tile([D + 1, R], F32, tag="p2s")
            p1_s = tail_pool.tile([D + 1, R], F32, tag="p1s")
            nc.vector.tensor_copy(out=p2_s, in_=p2_ps)
            nc.vector.tensor_copy(out=p1_s, in_=p1_ps)

            # S and R rows moved to partition base 0, then transposed to
            # per-row columns st/rt [128, C]
            sr = tail_pool.tile([1, 2 * R], F32, tag="sr")
            nc.vector.tensor_copy(out=sr[0:1, 0:R], in_=p2_s[D:D + 1, :])
            nc.vector.tensor_copy(out=sr[0:1, R:2 * R], in_=p1_s[D:D + 1, :])

            srt_ps = psum_acc.tile([128, 2 * C], F32, tag="p1")
            for rc in range(C):
                nc.tensor.transpose(
                    srt_ps[:, rc:rc + 1], sr[0:1, rc * 128:(rc + 1) * 128],
                    ident[0:1, 0:1],
                )
                nc.tensor.transpose(
                    srt_ps[:, C + rc:C + rc + 1],
                    sr[0:1, R + rc * 128:R + (rc + 1) * 128],
                    ident[0:1, 0:1],
                )
            # a = 0.1/den, b = 1/(S*den), den = 1 + 0.1*R
            den = tail_pool.tile([128, C], F32, tag="den")
            nc.vector.tensor_scalar(out=den, in0=srt_ps[:, C:2 * C],
                                    scalar1=0.1, scalar2=1.0,
                                    op0=ALU.mult, op1=ALU.add)
            a_t = tail_pool.tile([128, C], F32, tag="a")
            nc.vector.reciprocal(out=a_t, in_=den)
            nc.vector.tensor_scalar_mul(out=a_t, in0=a_t, scalar1=0.1)
            b_t = tail_pool.tile([128, C], F32, tag="b")
            nc.vector.tensor_mul(out=b_t, in0=srt_ps[:, 0:C], in1=den)
            nc.vector.reciprocal(out=b_t, in_=b_t)

            # transpose P1/P2 back to [128 rows, D] chunks
            o_ps = psum_acc.tile([128, 2 * C * D], F32, tag="p2")
            for rc in range(C):
                nc.tensor.transpose(
                    o_ps[:, rc * D:(rc + 1) * D],
                    p1_s[0:D, rc * 128:(rc + 1) * 128], ident[0:D, 0:D],
                )
                nc.tensor.transpose(
                    o_ps[:, (C + rc) * D:(C + rc + 1) * D],
                    p2_s[0:D, rc * 128:(rc + 1) * 128], ident[0:D, 0:D],
                )

            ot = tail_pool.tile([128, C, D], F32, tag="ot")
            t1 = tail_pool.tile([128, D], F32, tag="t1")
            t2 = tail_pool.tile([128, D], F32, tag="t2")
            for rc in range(C):
                nc.vector.tensor_scalar_mul(
                    out=t1, in0=o_ps[:, rc * D:(rc + 1) * D],
                    scalar1=a_t[:, rc:rc + 1])
                nc.vector.tensor_scalar_mul(
                    out=t2, in0=o_ps[:, (C + rc) * D:(C + rc + 1) * D],
                    scalar1=b_t[:, rc:rc + 1])
                nc.vector.tensor_add(out=ot[:, rc, :], in0=t1, in1=t2)

            nc.sync.dma_start(
                out=out_h[r0:r0 + R, :].rearrange("(s p) d -> p s d", p=128),
                in_=ot,
            )

    if split_waits:
        _split_multi_waits(nc)
    return nc


_NC_CACHE = {}

# test-harness knobs (the grading harness uses the defaults)
TRACE = False
LAST_RESULT = None


def _get_nc(pad_rows, ny):
    key = (pad_rows, ny)
    if key not in _NC_CACHE:
        _NC_CACHE[key] = build_nc(pad_rows, ny)
    return _NC_CACHE[key]


def kernel(x, y, Wq, bq, Wk, bk, Wv, bv):
    from concourse.bass_utils import run_bass_kernel_spmd

    x = np.ascontiguousarray(np.asarray(x, dtype=np.float32))
    y = np.ascontiguousarray(np.asarray(y, dtype=np.float32))
    Wq = np.asarray(Wq, np.float32)
    bq = np.asarray(bq, np.float32)
    Wk = np.asarray(Wk, np.float32)
    bk = np.asarray(bk, np.float32)
    Wv = np.asarray(Wv, np.float32)
    bv = np.asarray(bv, np.float32)

    n = x.shape[0]
    rows_per_core = (n + N_CORES - 1) // N_CORES
    pad_rows = ((rows_per_core + 127) // 128) * 128
    ny = y.shape[0]

    nc = _get_nc(pad_rows, ny)

    xp = np.zeros((N_CORES, pad_rows, S_IN), np.float32)
    for c in range(N_CORES):
        lo = c * rows_per_core
        hi = min(lo + rows_per_core, n)
        xp[c, 0:hi - lo] = x[lo:hi]

    common = {"y": y, "Wq": Wq, "bq": bq, "Wk": Wk, "bk": bk, "Wv": Wv, "bv": bv}
    in_maps = [{"xs": xp[c], **common} for c in range(N_CORES)]
    kwargs = {}
    if TRACE:
        import shutil, tempfile
        shutil.rmtree("/tmp/kern_trace", ignore_errors=True)
        kwargs = dict(trace=True, tmpdir="/tmp/kern_trace")
    res = run_bass_kernel_spmd(nc, in_maps, core_ids=list(range(N_CORES)), **kwargs)
    global LAST_RESULT
    LAST_RESULT = res

    out = np.empty((n, D), np.float32)
    for c in range(N_CORES):
        lo = c * rows_per_core
        hi = min(lo + rows_per_core, n)
        out[lo:hi] = res.results[c]["out"][0:hi - lo]
    return out
